# revision 1
# baseline (speedup 1.0000x reference)
"""MHA on 8 TRN2 cores: (batch x q-token-half) sharding, all 16 heads/core,
final y-half per core; int8 wire for x, Wq/Wk/Wv AND Wo (per-block scales),
fp16 compute. See memory notes for the cost model."""

import numpy as np

import concourse.bacc as bacc
import concourse.bass as bass
import concourse.mybir as mybir
import concourse.tile as tile
from concourse.bass_utils import run_bass_kernel_spmd

P = 128
B, N, DIM, H, DH = 4, 2048, 1024, 16, 64
SCALE = DH ** -0.5
KT8 = DIM // P         # 8 contraction tiles for projections
CT8 = DIM // P         # 8 c'-tiles (= head pairs, all 16 heads)
NQ = N // 2            # q tokens per core
KTN = N // P           # 16 key tiles
XC = NQ + 2 * N        # xall columns: [xq half | xk | xv]
SWO = NQ + 2 * N       # sall offset of W scales (after sq|sk|sv)
F32 = mybir.dt.float32
F16 = mybir.dt.float16
I8 = mybir.dt.int8
EXP = mybir.ActivationFunctionType.Exp


def _build(reps: int = 1, loop: bool = False):
    nc = bacc.Bacc("TRN2", target_bir_lowering=False, debug=False, num_devices=8)
    if loop:
        nreps = nc.dram_tensor("nreps", [1, 1], mybir.dt.int32, kind="ExternalInput")
    # xall cols: [xq half (NQ) | xk (N) | xv (N)], d-major int8
    xall = nc.dram_tensor("xall", [DIM, XC], I8, kind="ExternalInput")
    # sall: [sq half | sk | sv | swq swk swv (4*DIM each) | swo (4*DIM)]
    sall = nc.dram_tensor("sall", [1, SWO + 16 * DIM], F32, kind="ExternalInput")
    wall = nc.dram_tensor("wall", [3 * DIM, DIM], I8, kind="ExternalInput")
    wot = nc.dram_tensor("wot", [DIM, DIM], I8, kind="ExternalInput")
    bqs = nc.dram_tensor("bqs", [DIM], F32, kind="ExternalInput")
    ones = nc.dram_tensor("ones", [1, KTN * H], F16, kind="ExternalInput")
    y = nc.dram_tensor("y", [NQ, DIM], F16, kind="ExternalOutput")

    with tile.TileContext(nc) as tc:
        with (
            tc.tile_pool(name="const", bufs=1) as const_pool,
            tc.tile_pool(name="kt", bufs=1) as kt_pool,
            tc.tile_pool(name="vt", bufs=1) as v_pool,
            tc.tile_pool(name="qt", bufs=1) as qt_pool,
        ):
            bq_sb = const_pool.tile([P, CT8], F32)
            nc.sync.dma_start(bq_sb[:], bqs.ap().rearrange("(t p) -> p t", p=P))
            # per-token dequant scales, replicated across partitions
            sq_sb = const_pool.tile([P, NQ], F32, name="sqb")
            sk_sb = const_pool.tile([P, N], F32, name="skb")
            sv_sb = const_pool.tile([P, N], F32, name="svb")
            nc.sync.dma_start(sq_sb[:], sall.ap()[:, 0:NQ].to_broadcast((P, NQ)))
            nc.sync.dma_start(sk_sb[:],
                              sall.ap()[:, NQ:NQ + N].to_broadcast((P, N)))
            nc.sync.dma_start(sv_sb[:],
                              sall.ap()[:, NQ + N:NQ + 2 * N].to_broadcast((P, N)))
            kt_sb = kt_pool.tile([P, CT8, N], F16)
            v_sb = v_pool.tile([P, KTN, H, DH + 1], F16)
            # ones column of V_aug (softmax denominator weights)
            nc.sync.dma_start(v_sb[:, :, :, DH:DH + 1],
                              ones.ap().to_broadcast((P, KTN * H)))
            qt_sb = qt_pool.tile([P, CT8, NQ], F16)

            if loop:
                nr_sb = const_pool.tile([1, 1], mybir.dt.int32)
                nc.sync.dma_start(nr_sb[:], nreps.ap())
                rv = nc.values_load(nr_sb[:], min_val=1, max_val=100000,
                                    skip_runtime_bounds_check=True)
                with tc.For_i(0, rv, 1):
                    _emit_once(nc, tc, xall, sall, wall, wot, y,
                               bq_sb, sq_sb, sk_sb, sv_sb, kt_sb, v_sb, qt_sb)
            else:
                for _ in range(reps):
                    _emit_once(nc, tc, xall, sall, wall, wot, y,
                               bq_sb, sq_sb, sk_sb, sv_sb, kt_sb, v_sb, qt_sb)
    nc.compile()
    return nc


def _emit_once(nc, tc, xall, sall, wall, wot, y,
               bq_sb, sq_sb, sk_sb, sv_sb, kt_sb, v_sb, qt_sb):
    # ---------------- projections: K, V, Q --------------------------------
    with (
        tc.tile_pool(name="xin", bufs=2) as x_pool,
        tc.tile_pool(name="xfp", bufs=2) as xf_pool,
        tc.tile_pool(name="win", bufs=1) as w_pool,
        tc.tile_pool(name="wdq", bufs=1) as wdq_pool,
        tc.tile_pool(name="wsc", bufs=1) as wsc_pool,
        tc.tile_pool(name="pps", bufs=4, space="PSUM") as proj_ps,
    ):
        def load_w(ai):
            """DMA int8 W block ai of wall, dequant to fp16 (d-quarter scales)."""
            ws = wsc_pool.tile([P, 4, DIM], F32, tag="ws")
            o = SWO + ai * 4 * DIM
            nc.sync.dma_start(ws[:],
                              sall.ap()[:, o:o + 4 * DIM].to_broadcast((P, 4 * DIM)))
            wi = w_pool.tile([P, KT8, DIM], I8, tag="wi")
            nc.sync.dma_start(
                wi[:], wall.ap().rearrange("(a t p) m -> p a t m", p=P, a=3)[:, ai, :, :])
            wf = wdq_pool.tile([P, KT8, DIM], F16, tag="wf")
            for t in range(KT8):
                nc.vector.tensor_mul(wf[:, t, :], wi[:, t, :], ws[:, t // 2, :])
            return wf
        def load_x(c0, s_sb, s0):
            """DMA int8 [P, KT8, 1024] slab of xall cols c0:c0+1024, dequant."""
            xi = x_pool.tile([P, KT8, 1024], I8, tag="x")
            nc.sync.dma_start(
                xi[:], xall.ap().rearrange("(t p) n -> p t n", p=P)[:, :, c0:c0 + 1024])
            xf = xf_pool.tile([P, KT8, 1024], F16, tag="xf")
            for t in range(KT8):
                nc.vector.tensor_mul(xf[:, t, :], xi[:, t, :],
                                     s_sb[:, s0:s0 + 1024])
            return xf

        # --- K projection: kt_sb[p, m, n] = (Wk^T Xk^T)[m*128+p, n]
        wk_sb = load_w(1)
        for nh in range(2):
            xh = load_x(NQ + nh * 1024, sk_sb, nh * 1024)
            for m in range(CT8):
                for nb in range(2):
                    ps = proj_ps.tile([P, 512], F32)
                    for kk in range(KT8):
                        nc.tensor.matmul(ps[:], wk_sb[:, kk, m * P:(m + 1) * P],
                                         xh[:, kk, nb * 512:(nb + 1) * 512],
                                         start=(kk == 0), stop=(kk == KT8 - 1))
                    nabs = nh * 1024 + nb * 512
                    nc.any.tensor_copy(kt_sb[:, m, nabs:nabs + 512], ps[:])
        # --- V projection: v_sb[p, tt, h, d] = (Xv Wv^T)[tt*128+p, h*64+d]
        wv_sb = load_w(2)
        for nh in range(2):
            xh = load_x(NQ + N + nh * 1024, sv_sb, nh * 1024)
            for tl in range(8):
                tt = nh * 8 + tl
                for mh in range(2):
                    ps = proj_ps.tile([P, 512], F32)
                    for kk in range(KT8):
                        nc.tensor.matmul(ps[:], xh[:, kk, tl * P:(tl + 1) * P],
                                         wv_sb[:, kk, mh * 512:(mh + 1) * 512],
                                         start=(kk == 0), stop=(kk == KT8 - 1))
                    nc.any.tensor_copy(v_sb[:, tt, 8 * mh:8 * mh + 8, 0:DH], ps[:])
        # --- Q projection (scaled weights; bias added at eviction)
        wq_sb = load_w(0)
        xh = load_x(0, sq_sb, 0)
        for nb in range(2):
            for m in range(CT8):
                ps = proj_ps.tile([P, 512], F32)
                for kk in range(KT8):
                    nc.tensor.matmul(ps[:], wq_sb[:, kk, m * P:(m + 1) * P],
                                     xh[:, kk, nb * 512:(nb + 1) * 512],
                                     start=(kk == 0), stop=(kk == KT8 - 1))
                nc.vector.tensor_scalar_add(qt_sb[:, m, nb * 512:(nb + 1) * 512],
                                            ps[:], bq_sb[:, m:m + 1])

    # ---------------- attention + out-projection --------------------------
    with (
        tc.tile_pool(name="wo", bufs=1) as wo_pool,
        tc.tile_pool(name="pt", bufs=6) as p_pool,
        tc.tile_pool(name="ot", bufs=2) as ot_pool,
        tc.tile_pool(name="ysb", bufs=3) as y_pool,
        tc.tile_pool(name="rc", bufs=3) as r_pool,
        tc.tile_pool(name="rcb", bufs=3) as rb_pool,
        tc.tile_pool(name="sps", bufs=2, space="PSUM") as s_ps,
        tc.tile_pool(name="avps", bufs=2, space="PSUM") as av_ps,
        tc.tile_pool(name="yps", bufs=2, space="PSUM") as y_ps,
    ):
        swo_sb = wo_pool.tile([P, 4, DIM], F32, name="swob")
        nc.sync.dma_start(
            swo_sb[:],
            sall.ap()[:, SWO + 12 * DIM:SWO + 16 * DIM].to_broadcast((P, 4 * DIM)))
        wo_i8 = wo_pool.tile([P, CT8, DIM], I8, name="woi")
        nc.sync.dma_start(wo_i8[:], wot.ap().rearrange("(t p) m -> p t m", p=P))
        wo_sb = wo_pool.tile([P, CT8, DIM], F16, name="wof")
        for ct in range(CT8):
            nc.vector.tensor_mul(wo_sb[:, ct, :], wo_i8[:, ct, :],
                                 swo_sb[:, ct // 2, :])
        for qb in range(2):
            q0 = qb * 512
            ot_t = ot_pool.tile([P, CT8, 512], F16)
            for pr in range(CT8):
                avs = [av_ps.tile([P, 512], F32, tag="av", name=f"av{_h}")
                       for _h in range(2)]
                for kt in range(KTN):
                    ss = s_ps.tile([P, 2, 512], F32)
                    for hh in range(2):
                        p0 = hh * 64
                        nc.tensor.matmul(
                            ss[:, hh, :],
                            kt_sb[p0:p0 + 64, pr, kt * P:(kt + 1) * P],
                            qt_sb[p0:p0 + 64, pr, q0:q0 + 512],
                            start=True, stop=True)
                    p_t = p_pool.tile([P, 2, 512], F16)
                    nc.scalar.activation(p_t[:], ss[:], EXP)
                    for hh in range(2):
                        h = 2 * pr + hh
                        nc.tensor.matmul(avs[hh][0:DH + 1, :], v_sb[:, kt, h, :],
                                         p_t[:, hh, :], start=(kt == 0),
                                         stop=(kt == KTN - 1))
                for hh in range(2):
                    p0 = hh * 64
                    rc = r_pool.tile([1, 512], F32)
                    nc.vector.reciprocal(rc[:], avs[hh][DH:DH + 1, :])
                    rcb = rb_pool.tile([DH, 512], F32)
                    nc.gpsimd.partition_broadcast(rcb[:], rc[:])
                    nc.vector.tensor_mul(ot_t[p0:p0 + 64, pr, :],
                                         avs[hh][0:DH, :], rcb[:])
            for tt in range(4):
                y_t = y_pool.tile([P, DIM], F16)
                for eb in range(2):
                    yp = y_ps.tile([P, 512], F32)
                    for ct in range(CT8):
                        nc.tensor.matmul(yp[:], ot_t[:, ct, tt * P:(tt + 1) * P],
                                         wo_sb[:, ct, eb * 512:(eb + 1) * 512],
                                         start=(ct == 0), stop=(ct == CT8 - 1))
                    nc.vector.tensor_copy(y_t[:, eb * 512:(eb + 1) * 512], yp[:])
                nc.sync.dma_start(y.ap()[q0 + tt * P:q0 + (tt + 1) * P, :], y_t[:])


_CACHE = {}


def _get_nc(reps: int = 1, loop: bool = False):
    key = (reps, loop)
    if key not in _CACHE:
        _CACHE[key] = _build(reps, loop)
    return _CACHE[key]


def _quant_i8(xt: np.ndarray):
    """Per-column (token) symmetric int8 quant of a [DIM, N] fp32 array."""
    s = np.abs(xt).max(axis=0, keepdims=True) / 127.0
    s = np.maximum(s, 1e-12).astype(np.float32)
    xi = np.rint(xt / s).clip(-127, 127).astype(np.int8)
    return xi, s


def _quant_w_i8(wt: np.ndarray):
    """int8 quant of transposed W [in_d, out_c], scale per (out-col,
    d-quarter); scales are rounded to fp16 BEFORE quantizing so the device
    dequant (int8 * fp16 scale) reproduces the host values exactly."""
    d, c = wt.shape
    wb = wt.reshape(4, d // 4, c)
    s16 = (np.abs(wb).max(axis=1, keepdims=True) / 127.0).astype(np.float16)
    s32 = np.maximum(s16.astype(np.float32), 1e-12)
    wi = np.rint(wb / s32).clip(-127, 127).astype(np.int8).reshape(d, c)
    return wi, np.ascontiguousarray(s16.reshape(1, 4 * c))


def _quant_w_i8(wt: np.ndarray):
    """int8 quant of transposed W [in_d, out_c], scale per (out-col,
    d-quarter)."""
    d, c = wt.shape
    wb = wt.reshape(4, d // 4, c)
    s = np.maximum(np.abs(wb).max(axis=1, keepdims=True) / 127.0,
                   1e-12).astype(np.float32)
    wi = np.rint(wb / s).clip(-127, 127).astype(np.int8).reshape(d, c)
    return wi, np.ascontiguousarray(s.reshape(1, 4 * c))


def make_in_maps(q, k, v, wq, bq, wk, bk, wv, bv, wo, bo):
    """Host-side sharding + quantization. Returns (in_maps, const_vec)."""
    q = np.asarray(q, np.float32); k = np.asarray(k, np.float32)
    v = np.asarray(v, np.float32)
    wq = np.asarray(wq, np.float32); wk = np.asarray(wk, np.float32)
    wv = np.asarray(wv, np.float32); wo = np.asarray(wo, np.float32)
    bq = np.asarray(bq, np.float32); bv = np.asarray(bv, np.float32)
    bo = np.asarray(bo, np.float32)

    xq_b, xk_b, xv_b = [], [], []
    for b in range(B):
        xq_b.append(_quant_i8(np.ascontiguousarray(q[b].T)))
        xk_b.append(_quant_i8(np.ascontiguousarray(k[b].T)))
        xv_b.append(_quant_i8(np.ascontiguousarray(v[b].T)))

    ones_arr = np.ones((1, KTN * H), np.float16)
    qq = _quant_w_i8(np.ascontiguousarray((wq * SCALE).T))
    kk2 = _quant_w_i8(np.ascontiguousarray(wk.T))
    vv = _quant_w_i8(np.ascontiguousarray(wv.T))
    oo = _quant_w_i8(np.ascontiguousarray(wo.T))
    wall_arr = np.concatenate([qq[0], kk2[0], vv[0]], axis=0)
    wsc = np.concatenate([qq[1], kk2[1], vv[1], oo[1]], axis=1)
    wot_arr = oo[0]
    bq_arr = np.ascontiguousarray(bq * SCALE)

    in_maps = []
    for c in range(8):
        b, r = c // 2, c % 2
        ts = slice(r * NQ, (r + 1) * NQ)
        in_maps.append({
            "xall": np.ascontiguousarray(np.concatenate(
                [xq_b[b][0][:, ts], xk_b[b][0], xv_b[b][0]], axis=1)),
            "sall": np.concatenate(
                [xq_b[b][1][:, ts], xk_b[b][1], xv_b[b][1], wsc], axis=1),
            "wall": wall_arr, "wot": wot_arr,
            "bqs": bq_arr, "ones": ones_arr,
        })
    const_vec = (bv.astype(np.float64) @ wo.astype(np.float64).T
                 + bo.astype(np.float64)).astype(np.float32)
    return in_maps, const_vec


def kernel(q, k, v, wq, bq, wk, bk, wv, bv, wo, bo):
    nc = _get_nc(1)
    in_maps, const_vec = make_in_maps(q, k, v, wq, bq, wk, bk, wv, bv, wo, bo)
    res = run_bass_kernel_spmd(nc, in_maps, core_ids=list(range(8)))
    out = np.empty((B, N, DIM), np.float32)
    for c in range(8):
        b, r = c // 2, c % 2
        out[b, r * NQ:(r + 1) * NQ] = (res.results[c]["y"].astype(np.float32)
                                       + const_vec)
    return out

